# revision 21
# baseline (speedup 1.0000x reference)
"""Trainium2 Bass kernel for nn_BiomechanicsLoss (masked quadratic-form loss).

Math per point: et = [u0, v1, w2, .5(u1+v0), .5(u2+w0), .5(w1+v2)],
q = et^T C et (C = f32 stiffness), loss = sqrt(sum_masked(q^2)) / count,
mask = gt_sdf < 1e-8.

q decomposes (A = 3x3 normal block of sym(C), d = Cs[3,3]/4) as
  q = alpha*(b1+b2+b3)^2 + g1*(b1^2+b2^2) + 1.0*b3^2 + x4+x5+x6
with b_i = a_i*s_i (host-applied scale), x_k = d*s_k'^2 (sqrt(d) host-
applied), and alpha solved so gamma3 == 1 exactly: b3^2 then joins the
shear squares in ONE scale-1 wide Square on ScalarE.

Distribution: pure data-parallel over points, 8 cores; host packs each
core's shard to bf16 [128, 10*J] with all constant scales folded into the
quantization; each core reduces to per-chunk (ssq, count) f32 columns which
the host combines (sqrt, divide).

Per chunk of width F (points per partition), engines (GpSimd does no
elementwise work: Pool compute measurably slows concurrent DVE SBUF
access):
  DMA   D2 [b1 b2 sd b3](4F) -> C[0:4F] first, then D1 [A..B..](6F) -> AB,
        both on the sync HWDGE ring so the FIFO drains in consumption order
  DVE   D2-only chain first: mask m = (sd<th) at 4x, s0 = b1+b2+b3;
        then s456 = AB[0:3F]+AB[3F:6F] -> C[4F:7F] ([b3|s456] contiguous),
        x6 = s6*s6, fold X[F:4F]+=Z, X[0:2F]+=X[2F:4F], q = X0+X1, qm = q*m
  ScalE Sign(th-sd) with accum -> count column (count = (N+sum)/2 on host);
        Z = [Sq(rg1*b12) | Sq(ra*s0)]; X[0:3F] = Sq([b3 s4 s5]);
        Sq(qm) with accum -> ssq column
All constant scales are folded into the host-side bf16 quantization
(sqrt(d) into the six shear components, a_i into b_i), so every device
square is either scale-free or a single scaled activation.
"""

import numpy as np

N = 4_194_304
NCORES = 8
N_LOCAL = N // NCORES  # 524288
P = 128
J = N_LOCAL // P  # 4096 points per partition
CHUNKS = [384, 1344, 1344, 1024]
NT = len(CHUNKS)
assert sum(CHUNKS) == J

THRESH = 1e-8


def _weights():
    vp, Ep = 0.4, 0.21
    Ci = np.zeros((6, 6), dtype=np.float64)
    Ci[0, 0] = 1 / Ep;  Ci[0, 1] = -vp / Ep; Ci[0, 2] = -vp / Ep
    Ci[1, 0] = -vp / Ep; Ci[1, 1] = 1 / Ep;  Ci[1, 2] = -vp / Ep
    Ci[2, 0] = -vp;      Ci[2, 1] = -vp;     Ci[2, 2] = 1 / Ep
    Ci[3, 3] = 2 * (1 + vp) / Ep
    Ci[4, 4] = Ci[3, 3]
    Ci[5, 5] = Ci[3, 3]
    # match reference: inverse computed in f64, cast to f32
    C = np.linalg.inv(Ci).astype(np.float32).astype(np.float64)
    Cs = 0.5 * (C + C.T)
    A = Cs[:3, :3]
    d = 0.25 * Cs[3, 3]
    A12, A13 = A[0, 1], A[0, 2]
    # alpha s.t. gamma3 == 1 (A13 == A23, A11 == A22 for these constants)
    alpha = A13 ** 2 / (A[2, 2] * A12 - A13 ** 2)
    a1 = np.sqrt(A12 / alpha)
    a3 = a1 * A13 / A12
    g1 = A[0, 0] / a1 ** 2 - alpha
    return dict(
        rd=float(np.sqrt(d)), a1=float(a1), a3=float(a3),
        rg1=float(np.sqrt(g1)), ra=float(np.sqrt(alpha)),
    )


_W = _weights()
_NC = None


def _build_nc():
    import concourse.bacc as bacc
    import concourse.mybir as mybir
    import concourse.tile as tile

    f32 = mybir.dt.float32
    bf16 = mybir.dt.bfloat16
    Sq = mybir.ActivationFunctionType.Square
    ALU = mybir.AluOpType
    rg1, ra = _W["rg1"], _W["ra"]

    nc = bacc.Bacc()
    packed = nc.dram_tensor("packed", [P, 10 * J], bf16, kind="ExternalInput")
    out = nc.dram_tensor("out", [P, 2 * NT], f32, kind="ExternalOutput")

    with tile.TileContext(nc) as tc:
        with (
            tc.tile_pool(name="io", bufs=3) as io,
            tc.tile_pool(name="mid", bufs=3) as mid,
            tc.tile_pool(name="stats", bufs=1) as stats_pool,
        ):
            stats = stats_pool.tile([P, 2 * NT], f32)

            # trigger the Square act-table load during the first DMA
            warm = stats_pool.tile([P, 2], bf16)
            nc.gpsimd.memset(warm, 0.0)
            nc.scalar.activation(warm, warm, Sq)
            # [P,1] threshold bias for the Sign activation
            thr = stats_pool.tile([P, 1], f32)
            nc.gpsimd.memset(thr, THRESH)

            c0 = 0
            for t, F in enumerate(CHUNKS):
                AB = io.tile([P, 6 * F], bf16, tag="AB")
                C = io.tile([P, 8 * F], bf16, tag="C")
                # D2 first: [b1 b2 | sd | b3] unblocks the mask/s0/x0
                # chain while D1 (the shear blocks) still streams; chunk 0
                # splits D2 so [b1 b2 | sd] lands even sooner
                if t == 0:
                    nc.sync.dma_start(out=C[:, 0:3 * F],
                                      in_=packed[:, c0 + 6 * F:c0 + 9 * F])
                    nc.sync.dma_start(out=C[:, 3 * F:4 * F],
                                      in_=packed[:, c0 + 9 * F:c0 + 10 * F])
                else:
                    nc.sync.dma_start(out=C[:, 0:4 * F],
                                      in_=packed[:, c0 + 6 * F:c0 + 10 * F])
                nc.sync.dma_start(out=AB[:], in_=packed[:, c0:c0 + 6 * F])
                c0 += 10 * F

                b12 = C[:, 0:2 * F]
                sd = C[:, 2 * F:3 * F]
                b3 = C[:, 3 * F:4 * F]

                X = mid.tile([P, 4 * F], bf16, tag="X")
                Z = mid.tile([P, 3 * F], bf16, tag="Z")
                junk = mid.tile([P, F], bf16, tag="junk")

                # --- D2-only chain (runs while D1 still streams) ---
                # mask at 4x (count comes from the Sign accum)
                m = mid.tile([P, F], bf16, tag="m")
                nc.vector.tensor_scalar(
                    out=m, in0=sd, scalar1=THRESH, scalar2=None,
                    op0=ALU.is_lt)
                # s0 = b1 + b2 + b3
                t0 = mid.tile([P, F], bf16, tag="t0")
                nc.vector.tensor_add(t0, C[:, 0:F], C[:, F:2 * F])
                s0 = mid.tile([P, F], bf16, tag="s0")
                nc.vector.tensor_add(s0, t0, b3)
                # signed mask on ScalarE: sign(th - sd), count accum
                nc.scalar.activation(
                    junk, sd, mybir.ActivationFunctionType.Sign,
                    scale=-1.0, bias=thr[:],
                    accum_out=stats[:, NT + t:NT + t + 1])
                # z1 z2 and x0 = (ra*s0)^2
                nc.scalar.activation(Z[:, 0:2 * F], b12, Sq, scale=rg1)
                nc.scalar.activation(Z[:, 2 * F:3 * F], s0, Sq, scale=ra)

                # --- D1-dependent chain ---
                # s456 -> C[4F:7F]: [b3|s4|s5|s6] one contiguous 4F run
                nc.vector.tensor_add(C[:, 4 * F:7 * F], AB[:, 0:3 * F],
                                     AB[:, 3 * F:6 * F])
                # X = [z3 x4 x5 x6]: z3/x4/x5 on ScalarE, x6 on DVE
                nc.scalar.activation(X[:, 0:3 * F], C[:, 3 * F:6 * F], Sq)
                nc.vector.tensor_mul(X[:, 3 * F:4 * F], C[:, 6 * F:7 * F],
                                     C[:, 6 * F:7 * F])

                # fold 7 terms -> q
                nc.vector.tensor_add(X[:, F:4 * F], X[:, F:4 * F], Z[:])
                nc.vector.tensor_add(X[:, 0:2 * F], X[:, 0:2 * F],
                                     X[:, 2 * F:4 * F])
                q = mid.tile([P, F], bf16, tag="q")
                nc.vector.tensor_add(q, X[:, 0:F], X[:, F:2 * F])

                # tail: qm = q*m, ssq accum
                nc.vector.tensor_mul(m, q, m)
                nc.scalar.activation(junk, m, Sq, accum_out=stats[:, t:t + 1])

            nc.sync.dma_start(out=out[:, :], in_=stats[:])

    nc.compile()
    return nc


def _get_nc():
    global _NC
    if _NC is None:
        _NC = _build_nc()
    return _NC


def _run(in_maps, trace=False, **kwargs):
    from concourse.bass_utils import run_bass_kernel_spmd

    nc = _get_nc()
    return run_bass_kernel_spmd(
        nc, in_maps, core_ids=list(range(NCORES)), trace=trace, **kwargs)


def _make_in_maps(grad_u, grad_v, grad_w, gt_sdf):
    import ml_dtypes
    bf = ml_dtypes.bfloat16

    grad_u = np.asarray(grad_u, dtype=np.float32)
    grad_v = np.asarray(grad_v, dtype=np.float32)
    grad_w = np.asarray(grad_w, dtype=np.float32)
    gt_sdf = np.asarray(gt_sdf, dtype=np.float32)
    rd = np.float32(_W["rd"]); a1 = np.float32(_W["a1"])
    a3 = np.float32(_W["a3"])

    in_maps = []
    for c in range(NCORES):
        sl = slice(c * N_LOCAL, (c + 1) * N_LOCAL)
        gu = grad_u[sl].reshape(P, J, 3)
        gv = grad_v[sl].reshape(P, J, 3)
        gw = grad_w[sl].reshape(P, J, 3)
        sd = gt_sdf[sl].reshape(P, J)
        parts = []
        off = 0
        for F in CHUNKS:
            s = slice(off, off + F)
            parts += [
                rd * gu[:, s, 1], rd * gu[:, s, 2], rd * gw[:, s, 1],  # A
                rd * gv[:, s, 0], rd * gw[:, s, 0], rd * gv[:, s, 2],  # B
                a1 * gu[:, s, 0], a1 * gv[:, s, 1],                    # b1 b2
                sd[:, s],
                a3 * gw[:, s, 2],                                      # b3
            ]
            off += F
        packed = np.ascontiguousarray(
            np.concatenate(parts, axis=1)).astype(bf)
        in_maps.append({"packed": packed})
    return in_maps


def _finalize(results):
    ssq = 0.0
    sgs = 0.0
    for res in results:
        st = np.asarray(res["out"], dtype=np.float64)
        ssq += st[:, :NT].sum()
        sgs += st[:, NT:].sum()
    # count columns hold sum(sign(th - sd)) = count_in - count_out
    cnt = 0.5 * (N + sgs)
    return np.float32(np.sqrt(ssq) / cnt)


def kernel(grad_u, grad_v, grad_w, gt_sdf):
    in_maps = _make_in_maps(grad_u, grad_v, grad_w, gt_sdf)
    res = _run(in_maps, trace=False)
    return _finalize(res.results)
